# revision 45
# baseline (speedup 1.0000x reference)
"""Transformer decoder layer (causal self-attn + cross-attn + FFN, 3 post-LNs)
on 8 Trainium2 NeuronCores — single-collective design.

Sharding: 2-way data parallel (batch) x 4-way within each batch group.
  core c: batch g = c // 4, rank r = c % 4; chunk = tokens [r*512, (r+1)*512).
  - self-attention: tensor parallel over heads (4 of 16 per core),
    ReduceScatter after the output projection -> each core holds its
    512-token chunk of the attention output (the ONLY collective).
  - LN1/LN2/LN3: chunk-local.
  - cross-attention: sequence parallel — all 16 heads for the own 512-token
    query chunk; encoder K/V computed full-width on every core (overlaps
    the ReduceScatter window). No collective needed.
  - FFN: chunk-local with full weights (w1/w2 streamed/parked in slots
    vacated by earlier phases). No collective needed.
  - output: each core writes its own [512, E] chunk; host concatenates.

Host passes x0 and enc pre-transposed ([E, S] feature-major) so no DMA
transposes are needed; x1/x2 chunk transposes are done on the PE.

Attention: scores [128 k, 512 q] blocks, exp via scalar engine, softmax
along k with a ones-column rowsum folded into the o-eviction scale;
probability tiles are streamed (4-deep pool) into the PV accumulation.
"""

import numpy as np
import ml_dtypes

import concourse.bass as bass
import concourse.bacc as bacc
import concourse.tile as tile
from concourse import mybir
from concourse import bass_utils
from concourse.masks import make_identity

F32 = mybir.dt.float32
BF16 = mybir.dt.bfloat16
AF = mybir.ActivationFunctionType
ALU = mybir.AluOpType

E = 1024
H = 16                  # total heads
SA_HPC = 4              # SA heads per core (16 / 4 ranks)
DK = 64
SA_QKV = SA_HPC * DK    # 256
EB = E // 128           # 8 E partition-blocks
FH = 4096               # FFN hidden


def _ts(i, n):
    return slice(i * n, (i + 1) * n)


def _pbcast(ap, p=128):
    """Broadcast a 1D DRAM AP across p partitions (partition step 0)."""
    return bass.AP(tensor=ap.tensor, offset=ap.offset, ap=[[0, p]] + list(ap.ap))


PHASES = ["null", "saqkv", "sa", "cakv", "rs1", "ln1", "caq", "ca",
          "ln2", "ffn1", "full"]


def build_decoder_nc(S: int, num_devices: int = 8, stop_after: str | None = None,
                     repeat: int = 1):
    assert S % 512 == 0
    nc = bacc.Bacc("TRN2", target_bir_lowering=False, debug=False,
                   num_devices=num_devices)

    G = 4
    CH = S // G            # chunk tokens per core

    din = {}

    def inp(name, shape, dt):
        din[name] = nc.dram_tensor(name, list(shape), dt, kind="ExternalInput")
        return din[name]

    inp("x0T_b", [E, S], BF16)
    inp("x0c_f", [CH, E], F32)
    inp("encT_b", [E, S], BF16)

    inp("sa_wq", [E, SA_QKV], BF16)
    inp("sa_wk", [E, SA_QKV], BF16)
    inp("sa_wv", [E, SA_QKV], BF16)
    inp("sa_wo", [SA_QKV, E], BF16)
    inp("sa_bq", [SA_QKV], F32)
    inp("sa_bk", [SA_QKV], F32)
    inp("sa_bv", [SA_QKV], BF16)
    inp("sa_bo4", [E], F32)          # bo / 4 (summed by the ReduceScatter)

    inp("ca_wq", [E, E], BF16)
    inp("ca_wk", [E, E], BF16)
    inp("ca_wv", [E, E], BF16)
    inp("ca_wo", [E, E], BF16)
    inp("ca_bq", [E], F32)
    inp("ca_bk", [E], F32)
    inp("ca_bv", [E], BF16)
    inp("ca_bo", [E], F32)

    inp("w1", [E, FH], BF16)
    inp("b1", [FH], F32)
    inp("w2", [FH, E], BF16)
    inp("b2", [E], F32)
    for i in (1, 2, 3):
        inp(f"ln{i}_g", [E], F32)
        inp(f"ln{i}_b", [E], F32)
    inp("cmask", [4, 128, 512], BF16)

    out = nc.dram_tensor("out", [CH, E], F32, kind="ExternalOutput")

    rg = [[0, 1, 2, 3], [4, 5, 6, 7]][: max(1, num_devices // 4)]
    if num_devices < 8:
        rg = [list(range(num_devices))]

    with tile.TileContext(nc) as tc:
        for rep in range(repeat):
            _emit(tc, din, out, S, rg, stop_after, sfx=f"_r{rep}")

    nc.compile()
    return nc


def _emit(tc, din, out, S, rg, stop_after=None, sfx=""):
    nc = tc.nc
    TB = S // 128          # k token blocks
    QT = S // 512          # SA query tiles
    CH = S // 4            # chunk tokens
    CB = CH // 128         # chunk token blocks

    def cut(phase):
        return stop_after == phase

    def finish():
        nc.sync.dma_start(out=out.ap(), in_=din["x0c_f"].ap())

    with (
        tc.tile_pool(name="const" + sfx, bufs=1) as const,
        tc.tile_pool(name="wpool" + sfx, bufs=1) as wpool,
        tc.tile_pool(name="bigA" + sfx, bufs=1) as bigA,
        tc.tile_pool(name="bigB" + sfx, bufs=1) as bigB,
        tc.tile_pool(name="bigC" + sfx, bufs=1) as bigC,
        tc.tile_pool(name="qk" + sfx, bufs=1) as qk_pool,
        tc.tile_pool(name="at" + sfx, bufs=3) as at_pool,
        tc.tile_pool(name="denp" + sfx, bufs=2) as denp,
        tc.tile_pool(name="ws" + sfx, bufs=2) as ws,
        tc.tile_pool(name="xtc" + sfx, bufs=1) as xtc_pool,
        tc.tile_pool(name="lnp" + sfx, bufs=2) as lnp,
        tc.tile_pool(name="stat" + sfx, bufs=8) as stat,
        tc.tile_pool(name="pp" + sfx, bufs=2, space="PSUM") as pp,
        tc.tile_pool(name="ps_s" + sfx, bufs=2, space="PSUM") as ps_s,
        tc.tile_pool(name="po" + sfx, bufs=2, space="PSUM") as po_pool,
        tc.tile_pool(name="dram" + sfx, bufs=1, space="DRAM") as dram,
    ):
        # ------------- critical input loads first (DMA-channel priority) ----
        x0T = bigA.tile([128, EB, S], BF16, tag="bigA", name="x0T" + sfx)
        nc.sync.dma_start(out=x0T[:, 0:4, :], in_=din["x0T_b"].ap()[0:512, :]
                          .rearrange("(eb p) s -> p eb s", p=128))
        nc.scalar.dma_start(out=x0T[:, 4:8, :], in_=din["x0T_b"].ap()[512:1024, :]
                            .rearrange("(eb p) s -> p eb s", p=128))
        encT = bigB.tile([128, EB, S], BF16, tag="bigB", name="encT" + sfx)
        nc.sync.dma_start(out=encT[:, 0:4, :], in_=din["encT_b"].ap()[0:512, :]
                          .rearrange("(eb p) s -> p eb s", p=128))
        nc.scalar.dma_start(out=encT[:, 4:8, :], in_=din["encT_b"].ap()[512:1024, :]
                            .rearrange("(eb p) s -> p eb s", p=128))

        sa_w = {}
        for nm in ("wq", "wk", "wv"):
            t = wpool.tile([128, EB, SA_QKV], BF16, tag=nm, name=f"sa_{nm}{sfx}")
            nc.sync.dma_start(out=t, in_=din[f"sa_{nm}"].ap().rearrange(
                "(eb p) m -> p eb m", p=128))
            sa_w[nm] = t
        sa_wo = wpool.tile([128, 2, E], BF16, tag="wo", name=f"sa_wo{sfx}")
        nc.sync.dma_start(out=sa_wo, in_=din["sa_wo"].ap().rearrange(
            "(j p) n -> p j n", p=128))

        # ---------------- constants ----------------
        ident = const.tile([128, 128], BF16)
        make_identity(nc, ident)
        eps_t = const.tile([128, 1], F32)
        nc.vector.memset(eps_t, 1e-12)
        cmask = const.tile([128, 4, 512], BF16)
        nc.sync.dma_start(out=cmask, in_=din["cmask"].ap().rearrange("i p q -> p i q"))

        _bcast_cache = {}

        def bcast(name, tag):
            if name in _bcast_cache:
                return _bcast_cache[name]
            t = const.tile([128, E], F32, name=f"bc_{name}{sfx}", tag=tag)
            nc.sync.dma_start(out=t, in_=_pbcast(din[name].ap()))
            _bcast_cache[name] = t
            return t

        def pp_bias(name, nj):
            t = const.tile([128, nj], F32, name=f"ppb_{name}{sfx}", tag=f"ppb_{name}")
            nc.sync.dma_start(out=t, in_=din[name].ap().rearrange("(j p) -> p j", p=128))
            return t

        sa_bq = pp_bias("sa_bq", 2)
        sa_bk = pp_bias("sa_bk", 2)
        ca_bq = pp_bias("ca_bq", 8)
        ca_bk = pp_bias("ca_bk", 8)
        b1_t = pp_bias("b1", FH // 128)

        ones_r = const.tile([1, 128], BF16, name=f"ones_r{sfx}", tag="ones_r")
        nc.vector.memset(ones_r, 1.0)

        def bvr(name, n):
            t = const.tile([1, n], BF16, name=f"bvr_{name}{sfx}", tag="bvr")
            nc.sync.dma_start(out=t[0:1, :], in_=_pbcast(din[name].ap(), p=1))
            return t

        # ---------------- DRAM scratch ----------------
        ar1_in = dram.tile([S, E], BF16, name="ar1_in" + sfx)
        rs1_out = dram.tile([CH, E], BF16, name="rs1_out" + sfx)
        x1c_d = dram.tile([CH, E], F32, name="x1c" + sfx)
        x2c_d = dram.tile([CH, E], F32, name="x2c" + sfx)

        # ---------------- helpers ----------------
        def proj_qk(xT, w, b, dst, nj, nt):
            # dst [128, nj, nt*512] feature-major = w.T @ xT + b
            for j in range(nj):
                for tt in range(nt):
                    ps = pp.tile([128, 512], F32, tag="pp")
                    for eb in range(EB):
                        nc.tensor.matmul(ps, w[:, eb, _ts(j, 128)],
                                         xT[:, eb, _ts(tt, 512)],
                                         start=(eb == 0), stop=(eb == EB - 1))
                    nc.scalar.activation(dst[:, j, _ts(tt, 512)], ps, AF.Identity,
                                         bias=b[:, j:j + 1])

        _wv_cache = {}

        def proj_v(xT, w_dram, bvr_t, dst, nh, vh=0, tbs=None):
            # emit ONE vw-wide chunk (heads vh*vw/64 ..) of the V projection,
            # for token blocks `tbs` (default all). Bias folded into PSUM via
            # a rank-1 ones (x) bias matmul (evict = Act Copy, no DVE).
            vw = min(512, nh * DK)
            if vh == 0 and tbs is None or (tbs and 0 in tbs and vh == 0):
                nc.vector.memset(dst[:, :, :, 64:65], 1.0)
            key = (id(dst), vh)
            if key in _wv_cache:
                wv = _wv_cache[key]
            else:
                wv = ws.tile([128, EB, vw], BF16, tag="ws")
                nc.sync.dma_start(out=wv, in_=w_dram[:, _ts(vh, vw)].rearrange(
                    "(eb p) m -> p eb m", p=128))
                _wv_cache[key] = wv
            for tb in (range(TB) if tbs is None else tbs):
                ps = pp.tile([128, vw], F32, tag="pp")
                nc.tensor.matmul(ps, ones_r[0:1, :], bvr_t[0:1, _ts(vh, vw)],
                                 start=True, stop=False)
                for eb in range(EB):
                    nc.tensor.matmul(ps, xT[:, eb, _ts(tb, 128)], wv[:, eb, :],
                                     start=False, stop=(eb == EB - 1))
                hs = slice(vh * (vw // DK), (vh + 1) * (vw // DK))
                nc.scalar.activation(
                    dst[:, tb, hs, 0:64],
                    ps.rearrange("p (h d) -> p h d", d=64), AF.Copy)

        def attention(qT, kT, v, oT, heads, qts, causal):
            # qT [128, nh/2, nqt*512]; kT [128, nh/2, S]; v [128, TB, nh, 65]
            # oT [128, nh/2, nqt*512] written directly (d-major): the PV matmul
            # keeps v stationary so out is [65, 512q]; row 64 is the softmax
            # denominator, divided out per q column via a partition broadcast.
            # qt-outer so per-tile consumers (out_proj) can interleave.
            for qt in qts:
                for h in heads:
                    hp = slice((h % 2) * 64, (h % 2) * 64 + 64)
                    j = h // 2
                    kb_max = min(TB, 4 * qt + 4) if causal else TB
                    po = po_pool.tile([65, 512], F32, tag="po")
                    for kp in range(kb_max // 2):
                        ps2 = ps_s.tile([128, 2, 512], F32, tag="ps_s")
                        for z in range(2):
                            nc.tensor.matmul(ps2[:, z, :],
                                             kT[hp, j, _ts(2 * kp + z, 128)],
                                             qT[hp, j, _ts(qt, 512)],
                                             start=True, stop=True)
                        at = at_pool.tile([128, 2, 512], BF16, tag="at")
                        nc.scalar.activation(at, ps2, AF.Exp, scale=0.125)
                        for z in range(2):
                            kb = 2 * kp + z
                            if causal and kb >= 4 * qt:
                                nc.vector.tensor_mul(at[:, z, :], at[:, z, :],
                                                     cmask[:, kb - 4 * qt, :])
                            nc.tensor.matmul(po, v[:, kb, h, :], at[:, z, :],
                                             start=(kb == 0),
                                             stop=(kb == kb_max - 1))
                    den = denp.tile([1, 512], F32, tag="den")
                    nc.vector.reciprocal(den, po[64:65, :])
                    denb = denp.tile([64, 512], F32, tag="denb")
                    nc.gpsimd.partition_broadcast(denb, den)
                    nc.vector.tensor_mul(oT[hp, j, _ts(qt, 512)], po[0:64, :], denb)

        def ln_tile(ld, i):
            # in-place layernorm of ld [128, E] with ln{i} params
            st = stat.tile([128, 2, 6], F32, tag="bnst")
            for sg in range(2):
                nc.vector.bn_stats(st[:, sg, :], ld[:, _ts(sg, 512)])
            mv = stat.tile([128, 2], F32, tag="bnmv")
            nc.vector.bn_aggr(mv, st)
            sd = stat.tile([128, 1], F32, tag="sd")
            nc.scalar.activation(sd, mv[:, 1:2], AF.Sqrt, bias=eps_t)
            rstd = stat.tile([128, 1], F32, tag="rstd")
            nc.vector.reciprocal(rstd, sd)
            nc.vector.tensor_scalar(ld, ld, mv[:, 0:1], rstd,
                                    ALU.subtract, ALU.mult)
            nc.vector.tensor_mul(ld, ld, bcast(f"ln{i}_g", "lng"))
            nc.vector.tensor_add(ld, ld, bcast(f"ln{i}_b", "lnb"))

        def transpose_chunk(xb_tb, xTc, tb):
            # xb_tb [128, E] bf16 token-major -> xTc[:, eb, tb*128:...]
            for eb in range(EB):
                pt = po_pool.tile([128, 128], BF16, tag="po")
                nc.tensor.transpose(pt, xb_tb[:, _ts(eb, 128)], ident)
                nc.vector.tensor_copy(xTc[:, eb, _ts(tb, 128)], pt)

        # ================= SA (TP over heads) =================
        if cut("null"):
            finish()
            return

        qT = qk_pool.tile([128, 2, S], BF16, tag="qT", name="sa_qT" + sfx)
        kT = qk_pool.tile([128, 2, S], BF16, tag="kT", name="sa_kT" + sfx)
        v = bigC.tile([128, TB, SA_HPC, 65], BF16, tag="bigC", name="sa_v" + sfx)
        proj_qk(x0T, sa_w["wq"], sa_bq, qT, 2, QT)
        proj_qk(x0T, sa_w["wk"], sa_bk, kT, 2, QT)
        proj_v(x0T, din["sa_wv"].ap(), bvr("sa_bv", SA_QKV), v, SA_HPC)  # vh=0 only

        if cut("saqkv"):
            finish()
            return

        oT = qk_pool.tile([128, 2, S], BF16, tag="oT", name="sa_oT" + sfx)
        bo4 = bcast("sa_bo4", "bo")
        for qt in range(QT):
            attention(qT, kT, v, oT, range(SA_HPC), [qt], causal=True)
            # out_proj partials (+ bo/4) for this tile -> ar1_in
            for tb in range(qt * 4, qt * 4 + 4):
                y = lnp.tile([128, E], BF16, tag="ln_bf")
                for nh2 in range(2):
                    ps = pp.tile([128, 512], F32, tag="pp")
                    for jj in range(2):
                        nc.tensor.matmul(ps, oT[:, jj, _ts(tb, 128)],
                                         sa_wo[:, jj, _ts(nh2, 512)],
                                         start=(jj == 0), stop=(jj == 1))
                    nc.vector.tensor_add(y[:, _ts(nh2, 512)], ps,
                                         bo4[:, _ts(nh2, 512)])
                nc.sync.dma_start(out=ar1_in[_ts(tb, 128), :], in_=y)

        if cut("sa"):
            finish()
            return

        # ========== encoder K/V (full width; overlaps RS1) ==========
        ekT = bigA.tile([128, EB, S], BF16, tag="bigA", name="ekT" + sfx)
        for j in range(EB):
            wk = ws.tile([128, EB, 128], BF16, tag="ws")
            nc.sync.dma_start(out=wk, in_=din["ca_wk"].ap()[:, _ts(j, 128)].rearrange(
                "(eb p) m -> p eb m", p=128))
            for tt in range(QT):
                ps = pp.tile([128, 512], F32, tag="pp")
                for eb in range(EB):
                    nc.tensor.matmul(ps, wk[:, eb, :], encT[:, eb, _ts(tt, 512)],
                                     start=(eb == 0), stop=(eb == EB - 1))
                nc.scalar.activation(ekT[:, j, _ts(tt, 512)], ps, AF.Identity,
                                     bias=ca_bk[:, j:j + 1])
        ev = bigC.tile([128, TB, H, 65], BF16, tag="bigC", name="ev" + sfx)
        bvr_ca = bvr("ca_bv", E)
        proj_v(encT, din["ca_wv"].ap(), bvr_ca, ev, H, vh=0)

        if cut("cakv"):
            finish()
            return

        nc.gpsimd.collective_compute(
            "ReduceScatter", ALU.add, replica_groups=rg,
            ins=[ar1_in.opt()], outs=[rs1_out.opt()])

        if cut("rs1"):
            finish()
            return

        # ========== LN1 (chunk) + x1 transpose ==========
        x1Tc = xtc_pool.tile([128, EB, CH], BF16, tag="xtc", name="x1Tc" + sfx)
        for tb in range(CB):
            ld = lnp.tile([128, E], F32, tag="ln_io")
            arb = lnp.tile([128, E], BF16, tag="ln_bf")
            nc.sync.dma_start(out=arb, in_=rs1_out[_ts(tb, 128), :])
            nc.sync.dma_start(out=ld, in_=din["x0c_f"].ap()[_ts(tb, 128), :])
            nc.vector.tensor_add(ld, ld, arb)
            ln_tile(ld, 1)
            nc.sync.dma_start(out=x1c_d[_ts(tb, 128), :], in_=ld)
            xb = lnp.tile([128, E], BF16, tag="ln_bf")
            nc.vector.tensor_copy(xb, ld)
            transpose_chunk(xb, x1Tc, tb)

        if cut("ln1"):
            finish()
            return

        # ========== CA (sequence parallel, all heads) ==========
        qTc = qk_pool.tile([128, EB, CH], BF16, tag="qT", name="ca_qTc" + sfx)
        for j in range(EB):
            wq = ws.tile([128, EB, 128], BF16, tag="ws")
            nc.sync.dma_start(out=wq, in_=din["ca_wq"].ap()[:, _ts(j, 128)].rearrange(
                "(eb p) m -> p eb m", p=128))
            ps = pp.tile([128, CH], F32, tag="pp")
            for eb in range(EB):
                nc.tensor.matmul(ps, wq[:, eb, :], x1Tc[:, eb, :],
                                 start=(eb == 0), stop=(eb == EB - 1))
            nc.scalar.activation(qTc[:, j, :], ps, AF.Identity,
                                 bias=ca_bq[:, j:j + 1])

        if cut("caq"):
            finish()
            return

        oTc = qk_pool.tile([128, EB, CH], BF16, tag="oT", name="ca_oTc" + sfx)
        attention(qTc, ekT, ev, oTc, range(8), [0], causal=False)
        proj_v(encT, din["ca_wv"].ap(), bvr_ca, ev, H, vh=1)
        wo0 = ws.tile([128, EB, 512], BF16, tag="ws")
        nc.sync.dma_start(out=wo0, in_=din["ca_wo"].ap()[:, 0:512].rearrange(
            "(j p) n -> p j n", p=128))
        wo1 = ws.tile([128, EB, 512], BF16, tag="ws")
        nc.sync.dma_start(out=wo1, in_=din["ca_wo"].ap()[:, 512:1024].rearrange(
            "(j p) n -> p j n", p=128))
        attention(qTc, ekT, ev, oTc, range(8, 16), [0], causal=False)

        if cut("ca"):
            finish()
            return

        # ========== CA out_proj + LN2 (chunk) + x2 transpose ==========
        ca_bo = bcast("ca_bo", "bo")
        x2Tc = xtc_pool.tile([128, EB, CH], BF16, tag="xtc", name="x2Tc" + sfx)
        for tb in range(CB):
            ld = lnp.tile([128, E], F32, tag="ln_io")
            nc.sync.dma_start(out=ld, in_=x1c_d[_ts(tb, 128), :])
            nc.vector.tensor_add(ld, ld, ca_bo)
            for nh2, wo in ((0, wo0), (1, wo1)):
                ps = pp.tile([128, 512], F32, tag="pp")
                for jj in range(EB):
                    nc.tensor.matmul(ps, oTc[:, jj, _ts(tb, 128)], wo[:, jj, :],
                                     start=(jj == 0), stop=(jj == EB - 1))
                nc.vector.tensor_add(ld[:, _ts(nh2, 512)], ld[:, _ts(nh2, 512)], ps)
            ln_tile(ld, 2)
            nc.sync.dma_start(out=x2c_d[_ts(tb, 128), :], in_=ld)
            xb = lnp.tile([128, E], BF16, tag="ln_bf")
            nc.vector.tensor_copy(xb, ld)
            transpose_chunk(xb, x2Tc, tb)

        if cut("ln2"):
            finish()
            return

        # ========== FFN (chunk-local, full weights) ==========
        # w2 parks in the slots vacated by ekT (bigA) and encT (bigB)
        w2a = bigA.tile([128, 16, E], BF16, tag="bigA", name="w2a" + sfx)
        nc.scalar.dma_start(out=w2a, in_=din["w2"].ap()[0:2048, :].rearrange(
            "(hb p) n -> p hb n", p=128))
        w2b = bigB.tile([128, 16, E], BF16, tag="bigB", name="w2b" + sfx)
        nc.scalar.dma_start(out=w2b, in_=din["w2"].ap()[2048:4096, :].rearrange(
            "(hb p) n -> p hb n", p=128))

        hT = bigC.tile([128, FH // 128, CH], BF16, tag="bigC", name="hT" + sfx)
        for hc in range(FH // 512):
            w1c = ws.tile([128, EB, 512], BF16, tag="ws")
            nc.sync.dma_start(out=w1c, in_=din["w1"].ap()[:, _ts(hc, 512)].rearrange(
                "(eb p) m -> p eb m", p=128))
            for hl in range(4):
                hb = hc * 4 + hl
                ps = pp.tile([128, 512], F32, tag="pp")
                for eb in range(EB):
                    nc.tensor.matmul(ps, w1c[:, eb, _ts(hl, 128)], x2Tc[:, eb, :],
                                     start=(eb == 0), stop=(eb == EB - 1))
                nc.scalar.activation(hT[:, hb, :], ps, AF.Relu,
                                     bias=b1_t[:, hb:hb + 1])

        if cut("ffn1"):
            finish()
            return

        b2 = bcast("b2", "bo")
        for tb in range(CB):
            ld = lnp.tile([128, E], F32, tag="ln_io")
            nc.sync.dma_start(out=ld, in_=x2c_d[_ts(tb, 128), :])
            nc.vector.tensor_add(ld, ld, b2)
            for nh2 in range(2):
                ps = pp.tile([128, 512], F32, tag="pp")
                for hb in range(16):
                    nc.tensor.matmul(ps, hT[:, hb, _ts(tb, 128)],
                                     w2a[:, hb, _ts(nh2, 512)],
                                     start=(hb == 0), stop=False)
                for hb in range(16):
                    nc.tensor.matmul(ps, hT[:, 16 + hb, _ts(tb, 128)],
                                     w2b[:, hb, _ts(nh2, 512)],
                                     start=False, stop=(hb == 15))
                nc.vector.tensor_add(ld[:, _ts(nh2, 512)], ld[:, _ts(nh2, 512)], ps)
            ln_tile(ld, 3)
            nc.sync.dma_start(out=out.ap()[_ts(tb, 128), :], in_=ld)


# ====================== host side ======================

def make_causal_masks():
    m = np.zeros((4, 128, 512), dtype=np.float32)
    pk = np.arange(128)[:, None]
    pq = np.arange(512)[None, :]
    for i in range(4):
        m[i] = (pk <= pq - 128 * i).astype(np.float32)
    return m.astype(ml_dtypes.bfloat16)


def shard_inputs(inputs, num_devices=8):
    bf = ml_dtypes.bfloat16
    f32 = np.float32
    G = 4
    cmask = make_causal_masks()
    inp = {k: np.asarray(v) for k, v in inputs.items()}
    S = inp["input"].shape[1]
    CH = S // G
    in_maps = []
    xT_c, encT_c = {}, {}
    for c in range(num_devices):
        g, r = c // G, c % G
        if g not in xT_c:
            xT_c[g] = np.ascontiguousarray(inp["input"][g].T.astype(bf))
            encT_c[g] = np.ascontiguousarray(inp["encoder_output"][g].T.astype(bf))
        qs = slice(r * SA_QKV, (r + 1) * SA_QKV)
        m = {
            "x0T_b": xT_c[g],
            "x0c_f": inp["input"][g][r * CH:(r + 1) * CH].astype(f32),
            "encT_b": encT_c[g],
            "sa_wq": inp["sa_wq"][:, qs].astype(bf),
            "sa_wk": inp["sa_wk"][:, qs].astype(bf),
            "sa_wv": inp["sa_wv"][:, qs].astype(bf),
            "sa_wo": inp["sa_wo"][qs, :].astype(bf),
            "sa_bq": inp["sa_bq"][qs].astype(f32),
            "sa_bk": inp["sa_bk"][qs].astype(f32),
            "sa_bv": inp["sa_bv"][qs].astype(bf),
            "sa_bo4": (inp["sa_bo"] / G).astype(f32),
            "ca_wq": inp["ca_wq"].astype(bf),
            "ca_wk": inp["ca_wk"].astype(bf),
            "ca_wv": inp["ca_wv"].astype(bf),
            "ca_wo": inp["ca_wo"].astype(bf),
            "ca_bq": inp["ca_bq"].astype(f32),
            "ca_bk": inp["ca_bk"].astype(f32),
            "ca_bv": inp["ca_bv"].astype(bf),
            "ca_bo": inp["ca_bo"].astype(f32),
            "w1": inp["ffn_w1"].astype(bf),
            "b1": inp["ffn_b1"].astype(f32),
            "w2": inp["ffn_w2"].astype(bf),
            "b2": inp["ffn_b2"].astype(f32),
            "cmask": cmask,
        }
        for i in (1, 2, 3):
            m[f"ln{i}_g"] = inp[f"ln{i}_g"].astype(f32)
            m[f"ln{i}_b"] = inp[f"ln{i}_b"].astype(f32)
        in_maps.append(m)
    return in_maps


_NC_CACHE = {}


def _get_nc(S):
    if S not in _NC_CACHE:
        _NC_CACHE[S] = build_decoder_nc(S)
    return _NC_CACHE[S]


def kernel(**inputs):
    x = np.asarray(inputs["input"])
    B, S, _ = x.shape
    nc = _get_nc(S)
    in_maps = shard_inputs(inputs)
    res = bass_utils.run_bass_kernel_spmd(nc, in_maps, core_ids=list(range(8)))
    outb = [np.concatenate([res.results[g * 4 + r]["out"] for r in range(4)], axis=0)
            for g in range(B)]
    return np.stack(outb, axis=0).astype(np.float32)
